# revision 13
# baseline (speedup 1.0000x reference)
"""Trainium2 Bass kernel for a single-head unscaled-softmax attention layer.

Reference computation (fp32):
    q = X @ Wq + bq ; k = X @ Wk + bk ; v = X @ Wv + bv        X: [B=4, N=2048, D=1024]
    out = softmax(q @ k^T, axis=-1) @ v                         (no 1/sqrt(d) scale)

Sharding: 8 cores = (batch b, sequence half h).  Each core computes attention
for its 1024 query rows against the full 2048 keys of its batch.  K/V
projections are NOT recomputed per pair: each core projects only its own 1024
rows of K^T and V and the halves are exchanged through a pair-wise DRAM
AllGather (replica groups [0,1][2,3][4,5][6,7]), eliminating ~30% of the
tensor-engine work.  After the gather both cores hold K/V in the same global
row order; attention is permutation-invariant over keys so the kernel stays
identical SPMD across all 8 cores (only the host-side query-row gather maps
each core's output back).

Per-core kernel, fp16/bf16 compute with fp32 psum accumulation:
  inputs arrive pre-cast: XT(own half)/W/bv fp16 (halves DMA + SBUF footprint
  and enables FWL weight loads; logits from fp16 q/k carry ~7e-3 abs error,
  well inside the 2e-2 gate)
  K-own: K^T[e, m_own] = Wk^T X^T + bk -> DRAM kin, pair AllGather -> kout
  V-own: V[m_own, e]   = X Wv + bv    -> DRAM vin, pair AllGather -> vout
         (bias added on DVE from a broadcast tile; no rank-1 bias matmuls)
  Q:     Q^T[e, n]     = Wq^T X^T + bq -> SBUF resident fp16
  (both collectives overlap with the V-own/Q projections on the PE)
  attention over the gathered 2048 keys:
    S^T[m,n] = K Q^T        (psum fp32, contract e)
    P^T      = exp(S^T)     (ACT; bf16 NOT fp16: logits reach ~60 and
                             exp(60)~1e26 overflows fp16; bf16 has fp32 range)
    V carries 8 appended ones-columns, so the softmax denominator rides the
    same matmuls: out[n, 0:1032] = P^T.T @ [V | 1] in three 344-wide psum
    chunks (the 512/512/2 split wasted a ~60-cycle matmul floor on the tiny
    denominator group)
    out     /= den          (DVE reciprocal + per-partition scale)

Schedule notes: X^T and the weights arrive host-packed partition-major so
each load is 128 contiguous 16KB DMA descriptors (strided loads generate
1-2KB descriptors and sustain only ~110 GB/s); DMA issue order is
first-needed-first (X^T, Wk, Wv, Wq; small consts go down the scalar queue
in parallel); warm-up matmuls on memset tiles cover the initial DMA window
and keep the HAM clock-gate warm; PSUM pools are split between the S and O
phases so the first O matmul doesn't wait on an S-phase bank; output blocks
are scaled and stored in three chunks (DVE/ACT alternating) so the final
store pipeline is short.
"""

import numpy as np

import concourse.bass as bass
import concourse.mybir as mybir
import concourse.tile as tile

B, N, D = 4, 2048, 1024
NCORES = 8
P = 128
NQ = N // 2          # query rows per core (== own key rows)
KD = D // P          # 8 contraction chunks over d_in
KE = D // P          # 8 chunks over d_out (e)
MC = N // P          # 16 key chunks of 128
MO = NQ // P         # 8 own key chunks
VE = D + 8           # V free width incl. 8 ones-columns for the denominator
OC = VE // 3         # 344: O-phase psum chunk width
FP = mybir.dt.float32
F16 = mybir.dt.float16
BF16 = mybir.dt.bfloat16
NWARM = 20           # warm-up matmuls covering the initial DMA window
GROUPS = [[0, 1], [2, 3], [4, 5], [6, 7]]


def _split_sync_waits(nc, max_waits=1):
    """Walrus codegen on this container accepts at most one sync-wait command
    per instruction; hoist excess waits onto NoOps injected just before the
    instruction on the same engine (engines execute in order, so blocking at
    the NoOp is equivalent)."""
    mb = mybir
    for fn in nc.m.functions:
        for bb in fn.blocks:
            insts = list(bb.instructions)
            new = []
            changed = False
            for inst in insts:
                si = getattr(inst, "sync_info", None)
                if si is not None and si.on_wait and len(si.on_wait) > max_waits:
                    waits = list(si.on_wait)
                    keep = waits[-max_waits:]
                    excess = waits[:-max_waits]
                    for i in range(0, len(excess), max_waits):
                        chunk = excess[i : i + max_waits]
                        nop = mb.InstNoOp(
                            name=f"{inst.name}-sw{i}", ins=[], outs=[],
                            engine=inst.engine,
                        )
                        nop.sync_info = mb.SyncInfo(on_wait=chunk, on_update=[])
                        new.append(nop)
                    inst.sync_info = mb.SyncInfo(
                        on_wait=keep, on_update=list(si.on_update or [])
                    )
                    changed = True
                new.append(inst)
            if changed:
                bb.instructions = new


def _emit_body(nc, tc, rep, params, consts, pools, dram, dedup=True):
    """One full attention computation for this core's shard."""
    XT, Wq, Wk, Wv, OUT = params
    bq_t, bk_t, bv_row, ones_row, bvb = consts
    v_pool, qt_pool, kt_pool = pools
    kin, vin, kout, vout = dram
    MM = nc.tensor.matmul
    ADD = mybir.AluOpType.add

    kt = [kt_pool.tile([P, N], F16, name=f"kt{rep}_{i}", tag="kt") for i in range(KE)]
    vt = [v_pool.tile([P, VE], BF16, name=f"vt{rep}_{i}", tag="vt") for i in range(MC)]
    qt = [qt_pool.tile([P, NQ], F16, name=f"qt{rep}_{i}", tag="qt") for i in range(KE)]

    # ---------------- projections (own rows only) ----------------
    # XT/W arrive host-packed partition-major ([P, KD, cols]) so each load is
    # 128 contiguous 16KB descriptors - strided (d p)->p d loads generate
    # 1-2KB descriptors and the DMA path only sustains ~110 GB/s at that
    # granularity (measured: 1MB xtb load took 16us).
    with (
        tc.tile_pool(name=f"w{rep}", bufs=3) as w_pool,
        tc.tile_pool(name=f"xt{rep}", bufs=1) as xt_pool,
        tc.tile_pool(name=f"kstg{rep}", bufs=3) as kstg_pool,
        tc.tile_pool(name=f"vstg{rep}", bufs=3) as vstg_pool,
        tc.tile_pool(name=f"pps{rep}", bufs=8, space="PSUM") as pps,
    ):
        # DMA issue order = first-needed first.
        xtb = xt_pool.tile([P, KD, NQ], F16, name=f"xtb{rep}", tag="xt")
        nc.sync.dma_start(xtb[:], XT[:])
        wkb = w_pool.tile([P, KD, D], F16, name=f"wkb{rep}", tag="w")
        nc.sync.dma_start(wkb[:], Wk[:])
        wvb = w_pool.tile([P, KD, D], F16, name=f"wvb{rep}", tag="w")
        nc.sync.dma_start(wvb[:], Wv[:])
        wqb = w_pool.tile([P, KD, D], F16, name=f"wqb{rep}", tag="w")
        nc.sync.dma_start(wqb[:], Wq[:])

        # Warm-up matmuls: keep the PE busy during the initial DMA window so
        # the HAM clock-gate is at 8/8 when real work starts.  Inputs are
        # memset tiles (no DMA dependency - the const loads queue behind the
        # multi-MB weight transfers); the psum tile is never read.
        if rep == 0:
            wmst = kstg_pool.tile([1, P], F16, name="wmst", tag="wmst")
            wmmv = kstg_pool.tile([1, 512], F16, name="wmmv", tag="wmmv")
            nc.vector.memset(wmst[:], 1.0)
            nc.vector.memset(wmmv[:], 0.5)
            warm = pps.tile([P, 512], FP, name="warm", tag="ps")
            for _ in range(NWARM):
                MM(warm[:], wmst[:], wmmv[:], start=True, stop=True)
            # bias-broadcast tile for V: bvb[p, e] = bv[e] via rank-1 matmul
            for eh in range(2):
                ps = pps.tile([P, 512], FP, name="mm", tag="ps")
                MM(ps[:], ones_row[:], bv_row[:, eh * 512 : (eh + 1) * 512],
                   start=True, stop=True)
                nc.vector.tensor_copy(bvb[:, eh * 512 : (eh + 1) * 512], ps[:])

        # K projection (own rows): K^T[e, m_own] -> kin -> pair AllGather.
        # The gather is split into two e-halves so the first collective
        # launches halfway through the K projection - the CC path sustains
        # only ~75 GB/s, and a single 2MB gather finishes barely before the
        # S phase needs it.
        for e in range(KE):
            kstg = kstg_pool.tile([P, NQ], F16, name=f"kstg{rep}", tag="kstg")
            for mbo in range(2):
                ps = pps.tile([P, 512], FP, name="mm", tag="ps")
                for d in range(KD):
                    MM(ps[:], wkb[:, d, e * P : (e + 1) * P],
                       xtb[:, d, mbo * 512 : (mbo + 1) * 512],
                       start=(d == 0), stop=(d == KD - 1))
                nc.vector.tensor_scalar_add(
                    kstg[:, mbo * 512 : (mbo + 1) * 512], ps[:], bk_t[:, e : e + 1]
                )
            h, eh2 = e // (KE // 2), e % (KE // 2)
            nc.scalar.dma_start(kin[h][eh2 * P : (eh2 + 1) * P, :], kstg[:])
            if eh2 == KE // 2 - 1:
                nc.gpsimd.collective_compute(
                    "AllGather", mybir.AluOpType.bypass, replica_groups=GROUPS,
                    ins=[kin[h][:]], outs=[kout[h][:]],
                )

        # V projection (own rows): V[m_own, e] -> vin -> pair AllGather
        for m in range(MO):
            vstg = vstg_pool.tile([P, D], BF16, name=f"vstg{rep}", tag="vstg")
            for eh in range(2):
                ps = pps.tile([P, 512], FP, name="mm", tag="ps")
                for d in range(KD):
                    MM(ps[:], xtb[:, d, m * P : (m + 1) * P],
                       wvb[:, d, eh * 512 : (eh + 1) * 512],
                       start=(d == 0), stop=(d == KD - 1))
                nc.vector.tensor_tensor(
                    vstg[:, eh * 512 : (eh + 1) * 512], ps[:],
                    bvb[:, eh * 512 : (eh + 1) * 512], ADD,
                )
            nc.scalar.dma_start(vin[m * P : (m + 1) * P, :], vstg[:])
        nc.gpsimd.collective_compute(
            "AllGather", mybir.AluOpType.bypass, replica_groups=GROUPS,
            ins=[vin[:]], outs=[vout[:]],
        )

        # Q projection: Q^T[e, n] -> SBUF resident fp16
        for e in range(KE):
            for mbo in range(2):
                ps = pps.tile([P, 512], FP, name="mm", tag="ps")
                for d in range(KD):
                    MM(ps[:], wqb[:, d, e * P : (e + 1) * P],
                       xtb[:, d, mbo * 512 : (mbo + 1) * 512],
                       start=(d == 0), stop=(d == KD - 1))
                nc.vector.tensor_scalar_add(
                    qt[e][:, mbo * 512 : (mbo + 1) * 512], ps[:], bq_t[:, e : e + 1]
                )

    # Gathered K/V -> SBUF (both halves in the pair's global row order; both
    # cores read identically - attention is permutation-invariant over keys).
    HK = NQ // 2
    for e in range(KE):
        h, eh2 = e // (KE // 2), e % (KE // 2)
        nc.sync.dma_start(kt[e][:, 0:NQ], kout[h][eh2 * P : (eh2 + 1) * P, :])
        nc.sync.dma_start(
            kt[e][:, NQ:N], kout[h][HK + eh2 * P : HK + (eh2 + 1) * P, :]
        )
    for m in range(MC):
        nc.sync.dma_start(vt[m][:, 0:D], vout[m * P : (m + 1) * P, :])
        nc.vector.memset(vt[m][:, D:VE], 1.0)

    # ---------------- attention ----------------
    with (
        tc.tile_pool(name=f"pt{rep}", bufs=MC) as pt_pool,
        tc.tile_pool(name=f"ostage{rep}", bufs=2) as ostage,
        tc.tile_pool(name=f"rec{rep}", bufs=4) as rec_pool,
        tc.tile_pool(name=f"sps{rep}", bufs=2, space="PSUM") as st_ps,
        tc.tile_pool(name=f"ops{rep}", bufs=6, space="PSUM") as out_ps,
    ):
        pts = [pt_pool.tile([P, NQ], BF16, name=f"pt{rep}_{i}", tag="pt")
               for i in range(MC)]
        # S^T = K Q^T, P~ = exp(S^T)
        for m in range(MC):
            for nh in range(2):
                st = st_ps.tile([P, 512], FP, name="st", tag="ps")
                for e in range(KE):
                    MM(st[:], kt[e][:, m * P : (m + 1) * P],
                       qt[e][:, nh * 512 : (nh + 1) * 512],
                       start=(e == 0), stop=(e == KE - 1))
                nc.scalar.activation(
                    pts[m][:, nh * 512 : (nh + 1) * 512], st[:],
                    mybir.ActivationFunctionType.Exp,
                )
        # out = P~^T [V | 1] in three 344-wide chunks; den = ones column
        for nh in range(2):
            for ns in range(4):
                ob = [out_ps.tile([P, 512], FP, name=f"o{j}", tag="ps")
                      for j in range(3)]
                for m in range(MC):
                    lh = pts[m][:, nh * 512 + ns * P : nh * 512 + (ns + 1) * P]
                    for j in range(3):
                        MM(ob[j][:, 0:OC], lh, vt[m][:, j * OC : (j + 1) * OC],
                           start=(m == 0), stop=(m == MC - 1))
                rec = rec_pool.tile([P, 1], FP, name="rec", tag="rec")
                nc.vector.reciprocal(rec[:], ob[2][:, D - 2 * OC : D - 2 * OC + 1])
                ost = ostage.tile([P, D], FP, name="ost", tag="ost")
                nrow = nh * 512 + ns * P
                # chunked finalize: scale chunk j, then DMA it while the next
                # chunk scales (middle chunk on ACT so DVE and ACT overlap)
                nc.vector.tensor_scalar_mul(ost[:, 0:OC], ob[0][:, 0:OC], rec[:])
                nc.scalar.dma_start(OUT[nrow : nrow + P, 0:OC], ost[:, 0:OC])
                nc.scalar.activation(
                    ost[:, OC : 2 * OC], ob[1][:, 0:OC],
                    mybir.ActivationFunctionType.Copy, scale=rec[:],
                )
                nc.scalar.dma_start(
                    OUT[nrow : nrow + P, OC : 2 * OC], ost[:, OC : 2 * OC]
                )
                nc.vector.tensor_scalar_mul(
                    ost[:, 2 * OC : D], ob[2][:, 0 : D - 2 * OC], rec[:]
                )
                nc.scalar.dma_start(
                    OUT[nrow : nrow + P, 2 * OC : D], ost[:, 2 * OC : D]
                )


def build_bass(split=True, reps=1):
    nc = bass.Bass(num_devices=NCORES)
    XT = nc.declare_dram_parameter("XT", [P, KD, NQ], F16, isOutput=False)
    Wq = nc.declare_dram_parameter("Wq", [P, KD, D], F16, isOutput=False)
    Wk = nc.declare_dram_parameter("Wk", [P, KD, D], F16, isOutput=False)
    Wv = nc.declare_dram_parameter("Wv", [P, KD, D], F16, isOutput=False)
    BQ = nc.declare_dram_parameter("bq_t", [P, KE], FP, isOutput=False)
    BK = nc.declare_dram_parameter("bk_t", [P, KE], FP, isOutput=False)
    BV = nc.declare_dram_parameter("bv_row", [1, D], F16, isOutput=False)
    ONESR = nc.declare_dram_parameter("ones_row", [1, P], F16, isOutput=False)
    OUT = nc.declare_dram_parameter("OUT", [NQ, D], FP, isOutput=True)

    kin = [nc.dram_tensor(f"kin{h}", [NQ // 2, NQ], F16) for h in range(2)]
    vin = nc.dram_tensor("vin", [NQ, D], BF16)
    kout = [nc.dram_tensor(f"kout{h}", [NQ, NQ], F16) for h in range(2)]
    vout = nc.dram_tensor("vout", [N, D], BF16)

    with tile.TileContext(nc) as tc:
        with (
            tc.tile_pool(name="misc", bufs=1) as misc,
            tc.tile_pool(name="kt", bufs=KE) as kt_pool,
            tc.tile_pool(name="vt", bufs=MC) as v_pool,
            tc.tile_pool(name="qt", bufs=KE) as qt_pool,
        ):
            bq_t = misc.tile([P, KE], FP, tag="bq")
            bk_t = misc.tile([P, KE], FP, tag="bk")
            bv_row = misc.tile([1, D], F16, tag="bv")
            ones_row = misc.tile([1, P], F16, tag="onr")
            bvb = misc.tile([P, D], BF16, tag="bvb")
            nc.scalar.dma_start(bv_row[:], BV[:])
            nc.scalar.dma_start(ones_row[:], ONESR[:])
            nc.scalar.dma_start(bq_t[:], BQ[:])
            nc.scalar.dma_start(bk_t[:], BK[:])

            params = (XT, Wq, Wk, Wv, OUT)
            consts = (bq_t, bk_t, bv_row, ones_row, bvb)
            pools = (v_pool, qt_pool, kt_pool)
            dram = (kin, vin, kout, vout)
            for rep in range(reps):
                _emit_body(nc, tc, rep, params, consts, pools, dram)

    if split:
        _split_sync_waits(nc)
    return nc


_CACHE = {}


def _get_runner(reps=1, donate=True):
    """Compile once; return fn(in_maps) -> list[dict] running SPMD on 8 cores.

    reps>1 repeats the whole kernel body inside the NEFF (used for timing:
    slope over reps isolates per-body device time from dispatch overhead).
    """
    key = (reps, donate)
    if key in _CACHE:
        return _CACHE[key]

    import jax
    from jax.experimental.shard_map import shard_map
    from jax.sharding import Mesh, PartitionSpec

    from concourse import bass2jax

    nc = build_bass(reps=reps)
    bass2jax.install_neuronx_cc_hook()

    partition_name = (
        nc.partition_id_tensor.name if nc.partition_id_tensor else None
    )
    in_names, out_names, out_avals, zero_outs = [], [], [], []
    for alloc in nc.m.functions[0].allocations:
        if not isinstance(alloc, mybir.MemoryLocationSet):
            continue
        name = alloc.memorylocations[0].name
        if alloc.kind == "ExternalInput":
            if name != partition_name:
                in_names.append(name)
        elif alloc.kind == "ExternalOutput":
            shape = tuple(alloc.tensor_shape)
            dtype = mybir.dt.np(alloc.dtype)
            out_names.append(name)
            out_avals.append(jax.core.ShapedArray(shape, dtype))
            zero_outs.append(np.zeros(shape, dtype))
    n_params = len(in_names)
    n_outs = len(out_avals)
    all_in_names = list(in_names) + list(out_names)
    if partition_name is not None:
        all_in_names.append(partition_name)
    donate_idx = tuple(range(n_params, n_params + n_outs))

    def _body(*args):
        operands = list(args)
        if partition_name is not None:
            operands.append(bass2jax.partition_id_tensor())
        outs = bass2jax._bass_exec_p.bind(
            *operands,
            out_avals=tuple(out_avals),
            in_names=tuple(all_in_names),
            out_names=tuple(out_names),
            lowering_input_output_aliases=(),
            sim_require_finite=True,
            sim_require_nnan=True,
            nc=nc,
        )
        return tuple(outs)

    devices = jax.devices()[:NCORES]
    mesh = Mesh(np.asarray(devices), ("core",))
    in_specs = (PartitionSpec("core"),) * (n_params + n_outs)
    out_specs = (PartitionSpec("core"),) * n_outs
    sharded = jax.jit(
        shard_map(
            _body, mesh=mesh, in_specs=in_specs, out_specs=out_specs,
            check_rep=False,
        ),
        donate_argnums=donate_idx if donate else (),
        keep_unused=True,
    )

    def run(in_maps):
        import jax as _jax

        per_core = [[np.asarray(m[name]) for name in in_names] for m in in_maps]
        concat_in = [
            np.concatenate([per_core[c][i] for c in range(NCORES)], axis=0)
            for i in range(n_params)
        ]
        concat_zero = [np.concatenate([z] * NCORES, axis=0) for z in zero_outs]
        outs = sharded(*concat_in, *concat_zero)
        outs = [np.asarray(o) for o in _jax.block_until_ready(outs)]
        results = []
        for c in range(NCORES):
            r = {}
            for i, name in enumerate(out_names):
                d0 = out_avals[i].shape[0]
                r[name] = outs[i][c * d0 : (c + 1) * d0]
            results.append(r)
        return results

    run.sharded = sharded
    run.n_params = n_params
    run.in_names = in_names
    run.zero_outs = zero_outs
    _CACHE[key] = run
    return run


def _in_maps(X, Wq, bq, Wk, bk, Wv, bv):
    X = np.asarray(X, np.float32)
    maps = []
    bq_t = np.ascontiguousarray(np.asarray(bq, np.float32).reshape(KE, P).T)
    bk_t = np.ascontiguousarray(np.asarray(bk, np.float32).reshape(KE, P).T)
    bv_row = np.ascontiguousarray(np.asarray(bv, np.float16).reshape(1, D))

    def pmajor(W):  # [KD*P, cols] -> [P, KD, cols] (partition-major pack)
        W = np.asarray(W, np.float16)
        return np.ascontiguousarray(W.reshape(KD, P, W.shape[1]).transpose(1, 0, 2))

    Wq = pmajor(Wq)
    Wk = pmajor(Wk)
    Wv = pmajor(Wv)
    for c in range(NCORES):
        b, h = c // 2, c % 2
        XT = pmajor(X[b, h * NQ : (h + 1) * NQ].T.astype(np.float16))
        maps.append(
            dict(XT=XT, Wq=Wq, Wk=Wk, Wv=Wv, bq_t=bq_t, bk_t=bk_t,
                 bv_row=bv_row, ones_row=np.ones((1, P), np.float16))
        )
    return maps


def kernel(X, Wq, bq, Wk, bk, Wv, bv):
    run = _get_runner()
    results = run(_in_maps(X, Wq, bq, Wk, bk, Wv, bv))
    out = np.empty((B, N, D), np.float32)
    for c in range(NCORES):
        b, h = c // 2, c % 2
        out[b, h * NQ : (h + 1) * NQ, :] = results[c]["OUT"]
    return out


# revision 19
# speedup vs baseline: 1.0194x; 1.0194x over previous
"""Trainium2 Bass kernel for a single-head unscaled-softmax attention layer.

Reference computation (fp32):
    q = X @ Wq + bq ; k = X @ Wk + bk ; v = X @ Wv + bv        X: [B=4, N=2048, D=1024]
    out = softmax(q @ k^T, axis=-1) @ v                         (no 1/sqrt(d) scale)

Sharding: 8 cores = (batch b, sequence half h).  Each core computes attention
for its 1024 query rows against the full 2048 keys of its batch.  K/V
projections are NOT recomputed per pair: each core projects only its own 1024
rows of K^T and V and the halves are exchanged through a pair-wise DRAM
AllGather (replica groups [0,1][2,3][4,5][6,7]), eliminating ~30% of the
tensor-engine work.  After the gather both cores hold K/V in the same global
row order; attention is permutation-invariant over keys so the kernel stays
identical SPMD across all 8 cores (only the host-side query-row gather maps
each core's output back).

Per-core kernel, fp16/bf16 compute with fp32 psum accumulation:
  inputs arrive pre-cast: XT(own half)/W/bv fp16 (halves DMA + SBUF footprint
  and enables FWL weight loads; logits from fp16 q/k carry ~7e-3 abs error,
  well inside the 2e-2 gate)
  K-own: K^T[e, m_own] = Wk^T X^T + bk -> DRAM kin, pair AllGather -> kout
  V-own: V[m_own, e]   = X Wv + bv    -> DRAM vin, pair AllGather -> vout
         (bias added on DVE from a broadcast tile; no rank-1 bias matmuls)
  Q:     Q^T[e, n]     = Wq^T X^T + bq -> SBUF resident fp16
  (both collectives overlap with the V-own/Q projections on the PE)
  attention over the gathered 2048 keys:
    S^T[m,n] = K Q^T        (psum fp32, contract e)
    P^T      = exp(S^T)     (ACT; bf16 NOT fp16: logits reach ~60 and
                             exp(60)~1e26 overflows fp16; bf16 has fp32 range)
    V carries 8 appended ones-columns, so the softmax denominator rides the
    same matmuls: out[n, 0:1032] = P^T.T @ [V | 1] in three 344-wide psum
    chunks (the 512/512/2 split wasted a ~60-cycle matmul floor on the tiny
    denominator group)
    out     /= den          (DVE reciprocal + per-partition scale)

Schedule notes: X^T and the weights arrive host-packed partition-major so
each load is 128 contiguous 16KB DMA descriptors (strided loads generate
1-2KB descriptors and sustain only ~110 GB/s); DMA issue order is
first-needed-first (X^T, Wk, Wv, Wq; small consts go down the scalar queue
in parallel); warm-up matmuls on memset tiles cover the initial DMA window
and keep the HAM clock-gate warm; PSUM pools are split between the S and O
phases so the first O matmul doesn't wait on an S-phase bank; output blocks
are scaled and stored in three chunks (DVE/ACT alternating) so the final
store pipeline is short.
"""

import numpy as np

import concourse.bass as bass
import concourse.mybir as mybir
import concourse.tile as tile

B, N, D = 4, 2048, 1024
NCORES = 8
P = 128
NQ = N // 2          # query rows per core (== own key rows)
KD = D // P          # 8 contraction chunks over d_in
KE = D // P          # 8 chunks over d_out (e)
MC = N // P          # 16 key chunks of 128
MO = NQ // P         # 8 own key chunks
VE = D + 8           # V free width incl. 8 ones-columns for the denominator
OC = VE // 3         # 344: O-phase psum chunk width
FP = mybir.dt.float32
F16 = mybir.dt.float16
BF16 = mybir.dt.bfloat16
NWARM = 14           # warm-up matmuls covering the initial DMA window
GROUPS = [[0, 1], [2, 3], [4, 5], [6, 7]]


def _split_sync_waits(nc, max_waits=1):
    """Walrus codegen on this container accepts at most one sync-wait command
    per instruction; hoist excess waits onto NoOps injected just before the
    instruction on the same engine (engines execute in order, so blocking at
    the NoOp is equivalent)."""
    mb = mybir
    for fn in nc.m.functions:
        for bb in fn.blocks:
            insts = list(bb.instructions)
            new = []
            changed = False
            for inst in insts:
                si = getattr(inst, "sync_info", None)
                if si is not None and si.on_wait and len(si.on_wait) > max_waits:
                    waits = list(si.on_wait)
                    keep = waits[-max_waits:]
                    excess = waits[:-max_waits]
                    for i in range(0, len(excess), max_waits):
                        chunk = excess[i : i + max_waits]
                        nop = mb.InstNoOp(
                            name=f"{inst.name}-sw{i}", ins=[], outs=[],
                            engine=inst.engine,
                        )
                        nop.sync_info = mb.SyncInfo(on_wait=chunk, on_update=[])
                        new.append(nop)
                    inst.sync_info = mb.SyncInfo(
                        on_wait=keep, on_update=list(si.on_update or [])
                    )
                    changed = True
                new.append(inst)
            if changed:
                bb.instructions = new


def _emit_body(nc, tc, rep, params, consts, pools, dram, dedup=True):
    """One full attention computation for this core's shard."""
    XT, Wq, Wk, Wv, OUT = params
    bq_t, bk_t, bv_row, ones_row, bvb = consts
    v_pool, qt_pool, kt_pool = pools
    kin, vin, kout, vout = dram
    MM = nc.tensor.matmul
    ADD = mybir.AluOpType.add

    kt = [kt_pool.tile([P, N], F16, name=f"kt{rep}_{i}", tag="kt") for i in range(KE)]
    vt = [v_pool.tile([P, VE], BF16, name=f"vt{rep}_{i}", tag="vt") for i in range(MC)]
    qt = [qt_pool.tile([P, NQ], F16, name=f"qt{rep}_{i}", tag="qt") for i in range(KE)]

    # ---------------- projections (own rows only) ----------------
    # XT/W arrive host-packed partition-major ([P, KD, cols]) so each load is
    # 128 contiguous 16KB descriptors - strided (d p)->p d loads generate
    # 1-2KB descriptors and the DMA path only sustains ~110 GB/s at that
    # granularity (measured: 1MB xtb load took 16us).
    with (
        tc.tile_pool(name=f"w{rep}", bufs=3) as w_pool,
        tc.tile_pool(name=f"xt{rep}", bufs=1) as xt_pool,
        tc.tile_pool(name=f"kstg{rep}", bufs=3) as kstg_pool,
        tc.tile_pool(name=f"vstg{rep}", bufs=3) as vstg_pool,
        tc.tile_pool(name=f"pps{rep}", bufs=8, space="PSUM") as pps,
    ):
        # DMA issue order = first-needed first, at the granularity the K
        # e-loop consumes: X^T mbo-halves (1MB) interleaved with e-major Wk
        # chunks (256KB) so the first psum group is gated on ~1.25MB, not
        # the full 4MB.  All slices stay per-partition contiguous (large
        # DMA descriptors).
        xtb = xt_pool.tile([P, 2, KD, 512], F16, name=f"xtb{rep}", tag="xt")
        wkb = w_pool.tile([P, KE, D], F16, name=f"wkb{rep}", tag="w")
        nc.sync.dma_start(xtb[:, 0], XT[:, 0])
        nc.sync.dma_start(wkb[:, 0], Wk[:, 0])
        nc.sync.dma_start(xtb[:, 1], XT[:, 1])
        for e in range(1, KE):
            nc.sync.dma_start(wkb[:, e], Wk[:, e])
        wvb = w_pool.tile([P, KD, D], F16, name=f"wvb{rep}", tag="w")
        nc.sync.dma_start(wvb[:], Wv[:])
        wqb = w_pool.tile([P, KD, D], F16, name=f"wqb{rep}", tag="w")
        nc.sync.dma_start(wqb[:], Wq[:])

        # Warm-up matmuls: keep the PE busy during the initial DMA window so
        # the HAM clock-gate is at 8/8 when real work starts.  Inputs are
        # memset tiles (no DMA dependency - the const loads queue behind the
        # multi-MB weight transfers); the psum tile is never read.
        if rep == 0:
            wmst = kstg_pool.tile([1, P], F16, name="wmst", tag="wmst")
            wmmv = kstg_pool.tile([1, 512], F16, name="wmmv", tag="wmmv")
            nc.vector.memset(wmst[:], 1.0)
            nc.vector.memset(wmmv[:], 0.5)
            warm = pps.tile([P, 512], FP, name="warm", tag="ps")
            for _ in range(NWARM):
                MM(warm[:], wmst[:], wmmv[:], start=True, stop=True)
            # bias-broadcast tile for V: bvb[p, e] = bv[e] via rank-1 matmul
            for eh in range(2):
                ps = pps.tile([P, 512], FP, name="mm", tag="ps")
                MM(ps[:], ones_row[:], bv_row[:, eh * 512 : (eh + 1) * 512],
                   start=True, stop=True)
                nc.vector.tensor_copy(bvb[:, eh * 512 : (eh + 1) * 512], ps[:])

        # K projection (own rows): K^T[e, m_own] -> kin -> pair AllGather.
        # The gather is split into two e-halves so the first collective
        # launches halfway through the K projection - the CC path sustains
        # only ~75 GB/s, and a single 2MB gather finishes barely before the
        # S phase needs it.
        for e in range(KE):
            kstg = kstg_pool.tile([P, NQ], F16, name=f"kstg{rep}", tag="kstg")
            for mbo in range(2):
                ps = pps.tile([P, 512], FP, name="mm", tag="ps")
                for d in range(KD):
                    MM(ps[:], wkb[:, e, d * P : (d + 1) * P],
                       xtb[:, mbo, d, :],
                       start=(d == 0), stop=(d == KD - 1))
                nc.vector.tensor_scalar_add(
                    kstg[:, mbo * 512 : (mbo + 1) * 512], ps[:], bk_t[:, e : e + 1]
                )
            h, eh2 = e // (KE // 2), e % (KE // 2)
            nc.scalar.dma_start(kin[h][eh2 * P : (eh2 + 1) * P, :], kstg[:])
            if eh2 == KE // 2 - 1:
                nc.gpsimd.collective_compute(
                    "AllGather", mybir.AluOpType.bypass, replica_groups=GROUPS,
                    ins=[kin[h][:]], outs=[kout[h][:]],
                )

        # V projection (own rows): V[m_own, e] -> vin -> pair AllGather,
        # split in two m-halves so the first V gather launches as soon as
        # half the rows are staged (the CC queue, at ~75 GB/s serialized
        # behind the K gathers, is the slow resource).  The LAST m-chunk is
        # deferred until after the Q projection: its matmuls then sit
        # between Q's last psum group and the first S matmul, hiding the
        # DVE bias-add latency of the final qt tile (~3us otherwise).
        def v_chunk(m):
            vstg = vstg_pool.tile([P, D], BF16, name=f"vstg{rep}", tag="vstg")
            for eh in range(2):
                ps = pps.tile([P, 512], FP, name="mm", tag="ps")
                for d in range(KD):
                    MM(ps[:], xtb[:, m // 4, d, (m % 4) * P : (m % 4 + 1) * P],
                       wvb[:, d, eh * 512 : (eh + 1) * 512],
                       start=(d == 0), stop=(d == KD - 1))
                nc.vector.tensor_tensor(
                    vstg[:, eh * 512 : (eh + 1) * 512], ps[:],
                    bvb[:, eh * 512 : (eh + 1) * 512], ADD,
                )
            h, m2 = m // (MO // 2), m % (MO // 2)
            nc.scalar.dma_start(vin[h][m2 * P : (m2 + 1) * P, :], vstg[:])
            if m2 == MO // 2 - 1:
                nc.gpsimd.collective_compute(
                    "AllGather", mybir.AluOpType.bypass, replica_groups=GROUPS,
                    ins=[vin[h][:]], outs=[vout[h][:]],
                )

        for m in range(MO - 1):
            v_chunk(m)

        # Q projection: Q^T[e, n] -> SBUF resident fp16
        for e in range(KE):
            for mbo in range(2):
                ps = pps.tile([P, 512], FP, name="mm", tag="ps")
                for d in range(KD):
                    MM(ps[:], wqb[:, d, e * P : (e + 1) * P],
                       xtb[:, mbo, d, :],
                       start=(d == 0), stop=(d == KD - 1))
                nc.vector.tensor_scalar_add(
                    qt[e][:, mbo * 512 : (mbo + 1) * 512], ps[:], bq_t[:, e : e + 1]
                )

        v_chunk(MO - 1)

    # Gathered K/V -> SBUF (both halves in the pair's global row order; both
    # cores read identically - attention is permutation-invariant over keys).
    HK = NQ // 2
    # first halves of every kt tile load before any second half: the S phase
    # walks m ascending, so its first 8 m-chunks touch only kt[:, 0:NQ]
    for e in range(KE):
        h, eh2 = e // (KE // 2), e % (KE // 2)
        nc.sync.dma_start(kt[e][:, 0:NQ], kout[h][eh2 * P : (eh2 + 1) * P, :])
    for e in range(KE):
        h, eh2 = e // (KE // 2), e % (KE // 2)
        nc.sync.dma_start(
            kt[e][:, NQ:N], kout[h][HK + eh2 * P : HK + (eh2 + 1) * P, :]
        )
    HM = MO // 2 * P  # 512: rows per core half inside each vout
    for m in range(MC):
        pair, mm4 = (m % MO) // (MO // 2), m % (MO // 2)
        base = (m // MO) * HM + mm4 * P
        nc.sync.dma_start(vt[m][:, 0:D], vout[pair][base : base + P, :])
        nc.vector.memset(vt[m][:, D:VE], 1.0)

    # ---------------- attention ----------------
    with (
        tc.tile_pool(name=f"pt{rep}", bufs=MC) as pt_pool,
        tc.tile_pool(name=f"ostage{rep}", bufs=2) as ostage,
        tc.tile_pool(name=f"rec{rep}", bufs=4) as rec_pool,
        tc.tile_pool(name=f"sps{rep}", bufs=2, space="PSUM") as st_ps,
        tc.tile_pool(name=f"ops{rep}", bufs=6, space="PSUM") as out_ps,
    ):
        pts = [pt_pool.tile([P, NQ], BF16, name=f"pt{rep}_{i}", tag="pt")
               for i in range(MC)]
        # S^T = K Q^T, P~ = exp(S^T)
        for m in range(MC):
            for nh in range(2):
                st = st_ps.tile([P, 512], FP, name="st", tag="ps")
                for e in range(KE):
                    MM(st[:], kt[e][:, m * P : (m + 1) * P],
                       qt[e][:, nh * 512 : (nh + 1) * 512],
                       start=(e == 0), stop=(e == KE - 1))
                nc.scalar.activation(
                    pts[m][:, nh * 512 : (nh + 1) * 512], st[:],
                    mybir.ActivationFunctionType.Exp,
                )
        # out = P~^T [V | 1] in three 344-wide chunks; den = ones column
        for nh in range(2):
            for ns in range(4):
                ob = [out_ps.tile([P, 512], FP, name=f"o{j}", tag="ps")
                      for j in range(3)]
                for m in range(MC):
                    lh = pts[m][:, nh * 512 + ns * P : nh * 512 + (ns + 1) * P]
                    for j in range(3):
                        MM(ob[j][:, 0:OC], lh, vt[m][:, j * OC : (j + 1) * OC],
                           start=(m == 0), stop=(m == MC - 1))
                rec = rec_pool.tile([P, 1], FP, name="rec", tag="rec")
                nc.vector.reciprocal(rec[:], ob[2][:, D - 2 * OC : D - 2 * OC + 1])
                ost = ostage.tile([P, D], FP, name="ost", tag="ost")
                nrow = nh * 512 + ns * P
                # chunked finalize: scale chunk j, then DMA it while the next
                # chunk scales (middle chunk on ACT so DVE and ACT overlap)
                nc.vector.tensor_scalar_mul(ost[:, 0:OC], ob[0][:, 0:OC], rec[:])
                nc.scalar.dma_start(OUT[nrow : nrow + P, 0:OC], ost[:, 0:OC])
                nc.scalar.activation(
                    ost[:, OC : 2 * OC], ob[1][:, 0:OC],
                    mybir.ActivationFunctionType.Copy, scale=rec[:],
                )
                nc.scalar.dma_start(
                    OUT[nrow : nrow + P, OC : 2 * OC], ost[:, OC : 2 * OC]
                )
                nc.vector.tensor_scalar_mul(
                    ost[:, 2 * OC : D], ob[2][:, 0 : D - 2 * OC], rec[:]
                )
                nc.scalar.dma_start(
                    OUT[nrow : nrow + P, 2 * OC : D], ost[:, 2 * OC : D]
                )


def build_bass(split=True, reps=1):
    nc = bass.Bass(num_devices=NCORES)
    XT = nc.declare_dram_parameter("XT", [P, 2, KD, 512], F16, isOutput=False)
    Wq = nc.declare_dram_parameter("Wq", [P, KD, D], F16, isOutput=False)
    Wk = nc.declare_dram_parameter("Wk", [P, KE, D], F16, isOutput=False)
    Wv = nc.declare_dram_parameter("Wv", [P, KD, D], F16, isOutput=False)
    BQ = nc.declare_dram_parameter("bq_t", [P, KE], FP, isOutput=False)
    BK = nc.declare_dram_parameter("bk_t", [P, KE], FP, isOutput=False)
    BV = nc.declare_dram_parameter("bv_row", [1, D], F16, isOutput=False)
    ONESR = nc.declare_dram_parameter("ones_row", [1, P], F16, isOutput=False)
    OUT = nc.declare_dram_parameter("OUT", [NQ, D], FP, isOutput=True)

    kin = [nc.dram_tensor(f"kin{h}", [NQ // 2, NQ], F16) for h in range(2)]
    vin = [nc.dram_tensor(f"vin{h}", [NQ // 2, D], BF16) for h in range(2)]
    kout = [nc.dram_tensor(f"kout{h}", [NQ, NQ], F16) for h in range(2)]
    vout = [nc.dram_tensor(f"vout{h}", [NQ, D], BF16) for h in range(2)]

    with tile.TileContext(nc) as tc:
        with (
            tc.tile_pool(name="misc", bufs=1) as misc,
            tc.tile_pool(name="kt", bufs=KE) as kt_pool,
            tc.tile_pool(name="vt", bufs=MC) as v_pool,
            tc.tile_pool(name="qt", bufs=KE) as qt_pool,
        ):
            bq_t = misc.tile([P, KE], FP, tag="bq")
            bk_t = misc.tile([P, KE], FP, tag="bk")
            bv_row = misc.tile([1, D], F16, tag="bv")
            ones_row = misc.tile([1, P], F16, tag="onr")
            bvb = misc.tile([P, D], BF16, tag="bvb")
            nc.scalar.dma_start(bv_row[:], BV[:])
            nc.scalar.dma_start(ones_row[:], ONESR[:])
            nc.scalar.dma_start(bq_t[:], BQ[:])
            nc.scalar.dma_start(bk_t[:], BK[:])

            params = (XT, Wq, Wk, Wv, OUT)
            consts = (bq_t, bk_t, bv_row, ones_row, bvb)
            pools = (v_pool, qt_pool, kt_pool)
            dram = (kin, vin, kout, vout)
            for rep in range(reps):
                _emit_body(nc, tc, rep, params, consts, pools, dram)

    if split:
        _split_sync_waits(nc)
    return nc


_CACHE = {}


def _get_runner(reps=1, donate=True):
    """Compile once; return fn(in_maps) -> list[dict] running SPMD on 8 cores.

    reps>1 repeats the whole kernel body inside the NEFF (used for timing:
    slope over reps isolates per-body device time from dispatch overhead).
    """
    key = (reps, donate)
    if key in _CACHE:
        return _CACHE[key]

    import jax
    from jax.experimental.shard_map import shard_map
    from jax.sharding import Mesh, PartitionSpec

    from concourse import bass2jax

    nc = build_bass(reps=reps)
    bass2jax.install_neuronx_cc_hook()

    partition_name = (
        nc.partition_id_tensor.name if nc.partition_id_tensor else None
    )
    in_names, out_names, out_avals, zero_outs = [], [], [], []
    for alloc in nc.m.functions[0].allocations:
        if not isinstance(alloc, mybir.MemoryLocationSet):
            continue
        name = alloc.memorylocations[0].name
        if alloc.kind == "ExternalInput":
            if name != partition_name:
                in_names.append(name)
        elif alloc.kind == "ExternalOutput":
            shape = tuple(alloc.tensor_shape)
            dtype = mybir.dt.np(alloc.dtype)
            out_names.append(name)
            out_avals.append(jax.core.ShapedArray(shape, dtype))
            zero_outs.append(np.zeros(shape, dtype))
    n_params = len(in_names)
    n_outs = len(out_avals)
    all_in_names = list(in_names) + list(out_names)
    if partition_name is not None:
        all_in_names.append(partition_name)
    donate_idx = tuple(range(n_params, n_params + n_outs))

    def _body(*args):
        operands = list(args)
        if partition_name is not None:
            operands.append(bass2jax.partition_id_tensor())
        outs = bass2jax._bass_exec_p.bind(
            *operands,
            out_avals=tuple(out_avals),
            in_names=tuple(all_in_names),
            out_names=tuple(out_names),
            lowering_input_output_aliases=(),
            sim_require_finite=True,
            sim_require_nnan=True,
            nc=nc,
        )
        return tuple(outs)

    devices = jax.devices()[:NCORES]
    mesh = Mesh(np.asarray(devices), ("core",))
    in_specs = (PartitionSpec("core"),) * (n_params + n_outs)
    out_specs = (PartitionSpec("core"),) * n_outs
    sharded = jax.jit(
        shard_map(
            _body, mesh=mesh, in_specs=in_specs, out_specs=out_specs,
            check_rep=False,
        ),
        donate_argnums=donate_idx if donate else (),
        keep_unused=True,
    )

    def run(in_maps):
        import jax as _jax

        per_core = [[np.asarray(m[name]) for name in in_names] for m in in_maps]
        concat_in = [
            np.concatenate([per_core[c][i] for c in range(NCORES)], axis=0)
            for i in range(n_params)
        ]
        concat_zero = [np.concatenate([z] * NCORES, axis=0) for z in zero_outs]
        outs = sharded(*concat_in, *concat_zero)
        outs = [np.asarray(o) for o in _jax.block_until_ready(outs)]
        results = []
        for c in range(NCORES):
            r = {}
            for i, name in enumerate(out_names):
                d0 = out_avals[i].shape[0]
                r[name] = outs[i][c * d0 : (c + 1) * d0]
            results.append(r)
        return results

    run.sharded = sharded
    run.n_params = n_params
    run.in_names = in_names
    run.zero_outs = zero_outs
    _CACHE[key] = run
    return run


def _in_maps(X, Wq, bq, Wk, bk, Wv, bv):
    X = np.asarray(X, np.float32)
    maps = []
    bq_t = np.ascontiguousarray(np.asarray(bq, np.float32).reshape(KE, P).T)
    bk_t = np.ascontiguousarray(np.asarray(bk, np.float32).reshape(KE, P).T)
    bv_row = np.ascontiguousarray(np.asarray(bv, np.float16).reshape(1, D))

    def pmajor(W):  # [KD*P, cols] -> [P, KD, cols] (partition-major pack)
        W = np.asarray(W, np.float16)
        return np.ascontiguousarray(W.reshape(KD, P, W.shape[1]).transpose(1, 0, 2))

    def pmajor_e(W):  # [KD*P, KE*P] -> [P, KE, KD*P] (e-major pack for Wk)
        W = np.asarray(W, np.float16)
        return np.ascontiguousarray(
            W.reshape(KD, P, KE, P).transpose(1, 2, 0, 3).reshape(P, KE, D)
        )

    Wq = pmajor(Wq)
    Wk = pmajor_e(Wk)
    Wv = pmajor(Wv)
    for c in range(NCORES):
        b, h = c // 2, c % 2
        XTp = pmajor(X[b, h * NQ : (h + 1) * NQ].T.astype(np.float16))
        XT = np.ascontiguousarray(
            XTp.reshape(P, KD, 2, 512).transpose(0, 2, 1, 3)
        )
        maps.append(
            dict(XT=XT, Wq=Wq, Wk=Wk, Wv=Wv, bq_t=bq_t, bk_t=bk_t,
                 bv_row=bv_row, ones_row=np.ones((1, P), np.float16))
        )
    return maps


def kernel(X, Wq, bq, Wk, bk, Wv, bv):
    run = _get_runner()
    results = run(_in_maps(X, Wq, bq, Wk, bk, Wv, bv))
    out = np.empty((B, N, D), np.float32)
    for c in range(NCORES):
        b, h = c // 2, c % 2
        out[b, h * NQ : (h + 1) * NQ, :] = results[c]["OUT"]
    return out
